# revision 2
# baseline (speedup 1.0000x reference)
"""GCN layer (segment-sum message passing) on 8 Trainium2 NeuronCores.

out = D_in^{-1/2} A D_out^{-1/2} X W + b, A given as an edge list.

Strategy (dst-sharded, gather-free):
  - dst nodes sharded 12500/core across 8 cores; edges partitioned by dst core
    and bucketed into aligned 48-dst windows, padded to whole 128-edge chunks.
    Chunk counts per window are the max over the 8 cores so one SPMD program
    serves all cores (per-core variation lives in the data).
  - The host lays out an edge-expanded message table (row slot (p, c) holds
    x[src] of that edge slot, bf16) so the device streams messages with plain
    contiguous DMA -- no per-edge descriptor generation (SWDGE) at all.
  - Per chunk, the scatter matrix P[e, j] = (dstoff[e] == j) * rsqrt(deg_out)
    is built in two batched DVE passes laid out [128, W, PB] (chunk-minor) so
    every operand is a step-1 bf16 stream and both passes run in 2x perf mode:
    is_equal against a materialized iota, then multiply by the per-edge scale.
  - Aggregation per window: psum[64f, 48d] += msgs[128e, 64f]^T @ P[128e, 48d]
    accumulated over the window's chunks; eight windows (384 cols = 3 output
    blocks) share one psum tile that the scalar engine copies (cast bf16)
    into that group's private agg tile (per-group tiles keep the Tile
    dependency tracker's last-writer records exact).
  - Final phase per 128-dst block, interleaved two groups behind the window
    loop: psum2[128d, 64] = agg_blk^T @ W (bf16), then one fused DVE op
    applies the rsqrt(deg_in) scaling and adds the bias; outputs are flushed
    to HBM every 10 blocks so the store overlaps the remaining aggregation.
All float math (rsqrt, scaling, matmuls) runs on device. The host only does
integer graph restructuring (sharding/bucketing/padding) and array layout
(row gather/expansion + dtype casts of the staged tables).
"""
import os
import sys

sys.path.insert(0, "/opt/trn_rl_repo")

import numpy as np

import concourse.bass as bass
import concourse.bacc as bacc
import concourse.mybir as mybir
from concourse.bass_utils import run_bass_kernel_spmd
from concourse.tile import TileContext

N_NODES = 100000
N_EDGES = 1200000
D = 64
NCORES = 8
PER = N_NODES // NCORES          # 12500 dst nodes per core
W = 48                           # dsts per window (= one-hot width)
NWIN = (PER + W - 1) // W        # 261 windows
GW = 8                           # windows per psum group (8*48 = 384 cols)
GCOLS = GW * W                   # 384 = 3 output blocks
NGRP = (NWIN + GW - 1) // GW     # 33 groups
AGGW = NGRP * GCOLS              # 12672
NBLK = AGGW // 128               # 99 output blocks of 128 dsts
PERPAD = AGGW                    # padded output rows
CHK = 128                        # edges per chunk
TB = 16                          # chunks per msgs DMA tile
PB = 64                          # chunks per P-build batch
FDELAY = 2                       # final-block emission lag (window groups)

F32 = mybir.dt.float32
BF16 = mybir.dt.bfloat16
I16 = mybir.dt.int16

BF16_NP = mybir.dt.np(BF16)

LAST_EXEC_NS = None
LAST_SIM_ROWS = None


def _prep(x, edge_index):
    """Integer graph restructuring + array layout. No float arithmetic."""
    src = edge_index[0].astype(np.int64)
    dst = edge_index[1].astype(np.int64)
    deg_out = np.bincount(src, minlength=N_NODES)
    deg_in = np.bincount(dst, minlength=N_NODES)

    # global 48-dst windows, dealt to (core, position) slots by descending
    # edge count so the per-position max over cores ~= the mean (minimal
    # chunk padding in the shared SPMD schedule)
    NGW = (N_NODES + W - 1) // W          # 2084 global windows
    gw = dst // W
    woff = dst - gw * W
    cnt_g = np.bincount(gw, minlength=NGW)
    order_w = np.argsort(-cnt_g, kind="stable")
    core_of = np.zeros(NGW, np.int64)
    pos_of = np.zeros(NGW, np.int64)
    core_of[order_w] = np.arange(NGW) % NCORES
    pos_of[order_w] = np.arange(NGW) // NCORES
    assert pos_of.max() < NWIN

    core = core_of[gw]
    win = pos_of[gw]

    key = core * NWIN + win
    order = np.argsort(key, kind="stable")
    kcnt = np.bincount(key, minlength=NCORES * NWIN).reshape(NCORES, NWIN)
    K = np.maximum(1, -(-kcnt.max(axis=0) // CHK)).astype(np.int64)  # [NWIN]
    base = np.zeros(NWIN + 1, np.int64)
    np.cumsum(K, out=base[1:])
    nchunks = int(base[NWIN])

    okey = key[order]
    bucket_start = np.searchsorted(okey, np.arange(NCORES * NWIN), side="left")
    rank = np.arange(N_EDGES) - bucket_start[okey]

    so_src = src[order]
    so_core = core[order]
    so_win = win[order]
    so_woff = woff[order]
    slot = (base[so_win] + rank // CHK) * CHK + rank % CHK  # [E] global slot

    x_b = np.ascontiguousarray(x).astype(BF16_NP)

    cores_data = []
    for c in range(NCORES):
        m = so_core == c
        sl = slot[m]
        tab = np.zeros((nchunks * CHK, D), BF16_NP)
        tab[sl] = x_b[so_src[m]]
        # layout [128, nchunks*64]: chunk ch occupies cols [64*ch, 64*ch+64)
        tab = np.ascontiguousarray(
            tab.reshape(nchunks, CHK, D).transpose(1, 0, 2).reshape(CHK, -1)
        )

        dstoff = np.full((CHK, nchunks), -1, np.int16)
        dego = np.ones((CHK, nchunks), np.int16)
        p_of = sl % CHK
        c_of = sl // CHK
        dstoff[p_of, c_of] = so_woff[m].astype(np.int16)
        dego[p_of, c_of] = np.minimum(
            np.maximum(deg_out[so_src[m]], 1), 32000
        ).astype(np.int16)

        # this core's global windows, local agg col -> global dst map
        gws = np.where(core_of == c)[0]
        poss = pos_of[gws]
        # local column pos*W + j  <->  global dst gw*W + j (j < window len)
        dv = np.ones(PERPAD, np.int64)
        loc_cols = []
        glo_dsts = []
        for g, p in zip(gws, poss):
            n = min(W, N_NODES - g * W)
            lc = p * W + np.arange(n)
            gd = g * W + np.arange(n)
            loc_cols.append(lc)
            glo_dsts.append(gd)
        loc_cols = np.concatenate(loc_cols)
        glo_dsts = np.concatenate(glo_dsts)
        dv[loc_cols] = np.maximum(deg_in[glo_dsts], 1)
        degi = np.ones((CHK, NBLK), np.int16)
        d_arr = np.arange(PERPAD)
        degi[d_arr % CHK, d_arr // CHK] = np.minimum(dv, 32000).astype(np.int16)

        cores_data.append(
            {"tab": tab, "dstoff": dstoff, "dego": dego, "degi": degi,
             "loc_cols": loc_cols, "glo_dsts": glo_dsts}
        )

    return {"K": K, "base": base, "nchunks": nchunks}, cores_data


def _build(struct):
    K = struct["K"]
    base = struct["base"]
    nchunks = struct["nchunks"]

    nc = bacc.Bacc("TRN2", target_bir_lowering=False)
    t_tab = nc.declare_dram_parameter("tab", [CHK, nchunks * D], BF16,
                                      isOutput=False)
    t_dstoff = nc.declare_dram_parameter("dstoff", [CHK, nchunks], I16,
                                         isOutput=False)
    t_dego = nc.declare_dram_parameter("dego", [CHK, nchunks], I16,
                                       isOutput=False)
    t_degi = nc.declare_dram_parameter("degi", [CHK, NBLK], I16,
                                       isOutput=False)
    t_w = nc.declare_dram_parameter("w", [D, D], F32, isOutput=False)
    t_bb = nc.declare_dram_parameter("bb", [CHK, D], F32, isOutput=False)
    t_out = nc.declare_dram_parameter("out", [PERPAD, D], F32, isOutput=True)

    with TileContext(nc) as tc:
        with (
            tc.tile_pool(name="const", bufs=1) as cp,
            tc.tile_pool(name="msgs", bufs=8) as mp,
            tc.tile_pool(name="p0", bufs=3) as p0p,
            tc.tile_pool(name="p1", bufs=3) as p1p,
            tc.tile_pool(name="psg", bufs=5, space="PSUM") as psg,
            tc.tile_pool(name="psf", bufs=3, space="PSUM") as psf,
        ):
            msgs_tiles = {}

            def get_msgs(ch):
                t0 = ch // TB
                if t0 not in msgs_tiles:
                    c0 = t0 * TB
                    cc = min(TB, nchunks - c0)
                    t = mp.tile([CHK, cc, D], BF16, tag="msgs")
                    nc.sync.dma_start(
                        out=t[:], in_=t_tab[:, c0 * D:(c0 + cc) * D])
                    msgs_tiles[t0] = t
                return msgs_tiles[t0], ch - t0 * TB

            # materialized iota first: iota_mat[p, j, c] = j for all (p, c)
            iota_i = cp.tile([CHK, W], mybir.dt.int32)
            nc.gpsimd.iota(iota_i[:], pattern=[[1, W]], base=0,
                           channel_multiplier=0)
            iota_bf = cp.tile([CHK, W], BF16)
            nc.vector.tensor_copy(iota_bf[:], iota_i[:])
            iota_mat = cp.tile([CHK, W, PB], BF16)
            nc.vector.tensor_copy(
                iota_mat[:], iota_bf[:].unsqueeze(2).broadcast_to([CHK, W, PB]))

            dstoff_i = cp.tile([CHK, nchunks], I16)
            nc.sync.dma_start(out=dstoff_i[:], in_=t_dstoff[:])
            dego_i = cp.tile([CHK, nchunks], I16)
            nc.sync.dma_start(out=dego_i[:], in_=t_dego[:])
            degi_i = cp.tile([CHK, NBLK], I16)
            nc.sync.dma_start(out=degi_i[:], in_=t_degi[:])
            w_f = cp.tile([D, D], F32)
            nc.sync.dma_start(out=w_f[:], in_=t_w[:])
            bb_sb = cp.tile([CHK, D], F32)
            nc.sync.dma_start(out=bb_sb[:], in_=t_bb[:])

            # batch-0 P deps only: first PB cols of dstoff/sout, so the first
            # is_equal unblocks fast; everything else is deferred
            c1 = min(4 * PB, nchunks)
            dstoff_bf = cp.tile([CHK, nchunks], BF16)
            nc.vector.tensor_copy(dstoff_bf[:, :c1], dstoff_i[:, :c1])
            tmpf = cp.tile([CHK, nchunks], F32)
            sout_bf = cp.tile([CHK, nchunks], BF16)
            nc.vector.tensor_copy(tmpf[:, :c1], dego_i[:, :c1])
            nc.scalar.activation(tmpf[:, :c1], tmpf[:, :c1],
                                 mybir.ActivationFunctionType.Sqrt)
            nc.vector.reciprocal(tmpf[:, :c1], tmpf[:, :c1])
            nc.vector.tensor_copy(sout_bf[:, :c1], tmpf[:, :c1])

            agg_t = [cp.tile([D, GCOLS], BF16, name=f"agg{g}")
                     for g in range(NGRP)]
            w_bf = cp.tile([D, D], BF16)
            sin_f = cp.tile([CHK, NBLK], F32)
            rest_done = [False]

            def emit_rest():
                if rest_done[0]:
                    return
                rest_done[0] = True
                if nchunks > c1:
                    nc.vector.tensor_copy(dstoff_bf[:, c1:], dstoff_i[:, c1:])
                    nc.vector.tensor_copy(tmpf[:, c1:], dego_i[:, c1:])
                    nc.scalar.activation(tmpf[:, c1:], tmpf[:, c1:],
                                         mybir.ActivationFunctionType.Sqrt)
                    nc.vector.reciprocal(tmpf[:, c1:], tmpf[:, c1:])
                    nc.vector.tensor_copy(sout_bf[:, c1:], tmpf[:, c1:])
                nc.vector.tensor_copy(w_bf[:], w_f[:])
                # per-dst scale sin = rsqrt(max(deg_in, 1))
                nc.vector.tensor_copy(sin_f[:], degi_i[:])
                nc.scalar.activation(sin_f[:], sin_f[:],
                                     mybir.ActivationFunctionType.Sqrt)
                nc.vector.reciprocal(sin_f[:], sin_f[:])
                if NWIN * W < AGGW:
                    off = NWIN * W - (NGRP - 1) * GCOLS
                    nc.vector.memset(agg_t[NGRP - 1][:, off:], 0.0)



            p_tiles = {}

            nbatch = (nchunks + PB - 1) // PB

            def build_p(b0):
                if b0 == 4:
                    emit_rest()
                c0 = b0 * PB
                cc = min(PB, nchunks - c0)
                im = iota_mat[:] if cc == PB else iota_mat[:, :, :cc]
                p0 = p0p.tile([CHK, W, cc], BF16, tag="p0")
                nc.vector.tensor_tensor(
                    out=p0[:],
                    in0=dstoff_bf[:, c0:c0 + cc].unsqueeze(1)
                        .broadcast_to([CHK, W, cc]),
                    in1=im,
                    op=mybir.AluOpType.is_equal,
                )
                p1 = p1p.tile([CHK, W, cc], BF16, tag="p1")
                nc.vector.tensor_tensor(
                    out=p1[:], in0=p0[:],
                    in1=sout_bf[:, c0:c0 + cc].unsqueeze(1)
                        .broadcast_to([CHK, W, cc]),
                    op=mybir.AluOpType.mult,
                )
                p_tiles[b0] = p1

            def get_p(ch):
                b0 = ch // PB
                if b0 not in p_tiles:
                    build_p(b0)
                # prefetch next batch so its build overlaps consumption
                if b0 + 1 < nbatch and (b0 + 1) not in p_tiles:
                    build_p(b0 + 1)
                return p_tiles[b0], ch - b0 * PB

            OFL = 10  # blocks per output flush
            out_t = [cp.tile([CHK, min(OFL, NBLK - s0 * OFL) * D], F32,
                             name=f"outsb{s0}")
                     for s0 in range((NBLK + OFL - 1) // OFL)]

            def flush_out(b0, b1):
                if b1 > b0:
                    nc.sync.dma_start(
                        out=t_out[:].rearrange("(p k) f -> p k f", p=CHK)
                            [:, b0:b1, :],
                        in_=out_t[b0 // OFL][:],
                    )

            def emit_final(b):
                ps2 = psf.tile([CHK, D], F32)
                nc.tensor.matmul(
                    ps2[:],
                    agg_t[b // 3][:, (b % 3) * CHK:(b % 3 + 1) * CHK],
                    w_bf[:], start=True, stop=True)
                nc.vector.scalar_tensor_tensor(
                    out=out_t[b // OFL][:, (b % OFL) * D:(b % OFL + 1) * D],
                    in0=ps2[:],
                    scalar=sin_f[:, b:b + 1], in1=bb_sb[:],
                    op0=mybir.AluOpType.mult, op1=mybir.AluOpType.add,
                )
                if (b + 1) % OFL == 0:
                    flush_out(b + 1 - OFL, b + 1)

            # aggregation: GW windows share one [64, 384] psum tile (= 3
            # output blocks); finals for group g are emitted FDELAY groups
            # behind its ACT copy.
            for g in range(NGRP):
                wlo = g * GW
                whi = min(wlo + GW, NWIN)
                ps = psg.tile([D, (whi - wlo) * W], F32)
                for w in range(wlo, whi):
                    off = (w - wlo) * W
                    kw = int(K[w])
                    for k in range(kw):
                        ch = int(base[w]) + k
                        mt, mslot = get_msgs(ch)
                        pt, pslot = get_p(ch)
                        nc.tensor.matmul(
                            ps[:, off:off + W],
                            mt[:, mslot, :], pt[:, :, pslot],
                            start=(k == 0), stop=(k == kw - 1),
                        )
                nc.scalar.activation(
                    agg_t[g][:, :(whi - wlo) * W], ps[:],
                    mybir.ActivationFunctionType.Copy,
                )
                if g == FDELAY:
                    emit_rest()
                if g >= FDELAY:
                    for b in range((g - FDELAY) * 3, (g - FDELAY) * 3 + 3):
                        emit_final(b)
            for b in range((NGRP - FDELAY) * 3, NBLK):
                emit_final(b)
            flush_out((NBLK // OFL) * OFL, NBLK)

    nc.finalize()
    return nc


def kernel(**inputs):
    global LAST_EXEC_NS
    x = np.ascontiguousarray(np.asarray(inputs["x"], dtype=np.float32))
    edge_index = np.asarray(inputs["edge_index"]).astype(np.int64)
    Wm = np.ascontiguousarray(np.asarray(inputs["W"], dtype=np.float32))
    b = np.asarray(inputs["b"], dtype=np.float32).reshape(-1)

    struct, cores_data = _prep(x, edge_index)
    nc = _build(struct)

    bb = np.ascontiguousarray(np.tile(b[None, :], (CHK, 1)).astype(np.float32))
    in_maps = []
    for c in range(NCORES):
        cd = cores_data[c]
        in_maps.append({
            "tab": cd["tab"], "dstoff": cd["dstoff"], "dego": cd["dego"],
            "degi": cd["degi"], "w": Wm, "bb": bb,
        })

    if os.environ.get("GCN_SIM"):
        import concourse.bass_interp as bass_interp
        ncores_sim = int(os.environ.get("GCN_SIM_CORES", "1"))
        sim = bass_interp.MultiCoreSim(nc, ncores_sim)
        for c in range(ncores_sim):
            for k, v in in_maps[c].items():
                sim.cores[c].tensor(k)[:] = v
        sim.simulate()
        results = [{"out": np.array(sim.cores[c].mem_tensor("out"))}
                   for c in range(ncores_sim)]
        LAST_EXEC_NS = None
        out_full = np.zeros((N_NODES, D), np.float32)
        rows = []
        for c in range(ncores_sim):
            o = results[c]["out"]
            o = o.reshape(CHK, NBLK, D).transpose(1, 0, 2).reshape(PERPAD, D)
            out_full[cores_data[c]["glo_dsts"]] = o[cores_data[c]["loc_cols"]]
            rows.append(cores_data[c]["glo_dsts"])
        global LAST_SIM_ROWS
        LAST_SIM_ROWS = np.concatenate(rows)
        return out_full

    trace = bool(os.environ.get("GCN_TRACE"))
    res = run_bass_kernel_spmd(nc, in_maps, list(range(NCORES)), trace=trace)
    LAST_EXEC_NS = res.exec_time_ns
    out_full = np.zeros((N_NODES, D), np.float32)
    for c in range(NCORES):
        o = res.results[c]["out"]  # [PERPAD, 64], row r = p*NBLK + k, d=128k+p
        o = o.reshape(CHK, NBLK, D).transpose(1, 0, 2).reshape(PERPAD, D)
        out_full[cores_data[c]["glo_dsts"]] = o[cores_data[c]["loc_cols"]]
    return out_full
